# revision 33
# baseline (speedup 1.0000x reference)
"""Trainium2 Bass kernel for the GraphicalBranch GNN message-passing problem.

Math (equivalent to the reference):
  - Per-sample graphs are fully connected WITH self-loops over the nc2=28
    pair-nodes, so segment_sum(x[src], dst) == broadcast of the per-sample
    row-sum S[b] = sum_r x[b, r, :].
  - The final key-matching gather h[rows] commutes with the row-wise linear
    layer, so only the 10 gathered rows per sample are pushed through W_self:
        out[row] = relu(xg[row] @ W_self + (S[b(row)] @ W_nbr + b))
  - rows are computed on host from slicing_tensor/object_pairs (pure index
    arithmetic), exactly as the reference's LUT does.

Device-side structure (per core: 128 samples, 3584 x-rows, 1280 out rows):
  - Output rows are regrouped into 10 tiles of 128 rows keyed (t = h*5+rp):
    partition m of tile t holds relation r=2*rp+m//64 of sample s=64h+m%64,
    which makes the neighbor term A[s] PARTITION-ALIGNED per tile: a plain
    DVE add. No expansion one-hots.
  - W_self matmuls run fp8 DoubleRow: each output tile is 2 matmuls with a
    fused 256-deep contraction (kt-pairs), ~1.5x PE throughput.
  - Every W_self PSUM group is stashed to SBUF (bf16) the moment it closes
    (no dependency on A), so the 5 PSUM "mains" banks recycle early and the
    finish (z + A -> relu -> store) runs in cheap bf16 2x-mode ops.
  - Samples run in 2 halves of 64 (2 chunks of 32); aggregation pipelines
    through a bufs=3 PSUM chain pool (psS pair -> psTr -> psA per half);
    every bank is written, read once, freed - no bank read/write overlap.
  - Aggregation: one-hot G as lhsT, x as rhs; the two chunks of a half run
    as column-tiled concurrent matmuls (tile_position 0/32).
  - x/W_nbr travel bf16 (accuracy: they feed the dominant S@W_nbr term);
    xgT/W_self travel fp8 e4m3 (<1% absmax error). Output bf16, host
    upcasts. b is folded into the A copy via a partition-broadcast add.
  - 8 input DMAs = 8 DMA semaphore lanes: no issue-order inversion.
"""

import numpy as np
import ml_dtypes

# ---- problem constants (hardcoded; kernel.py must be self-contained) ----
B = 1024          # samples
NOBJ = 8          # objects per sample
NC2 = 28          # pair-nodes per sample
MAXR = 10         # relations per sample
D = 512           # feature dim
NCORES = 8
BL = B // NCORES          # 128 samples per core
RL = BL * NC2             # 3584 x-rows per core
ML = BL * MAXR            # 1280 output rows per core
KT = D // 128             # 4 contraction tiles
NH = 2                    # sample halves per core
HS = BL // NH             # 64 samples per half
NCH = 2                   # chunks per half
CS = HS // NCH            # 32 samples per chunk
NJ = 7                    # 128-row tiles per chunk (896 rows)
NT = 10                   # output tiles per core
NPRE = 5                  # xgT tiles prefetched before x

BF16 = ml_dtypes.bfloat16
FP8 = ml_dtypes.float8_e4m3

_compiled = None


def _build_bass():
    import concourse.bacc as bacc
    import concourse.bass as bass
    import concourse.mybir as mybir
    from concourse import tile

    f32 = mybir.dt.float32
    bf16 = mybir.dt.bfloat16
    fp8 = mybir.dt.float8e4
    Relu = mybir.ActivationFunctionType.Relu
    Add = mybir.AluOpType.add
    DR = mybir.MatmulPerfMode.DoubleRow

    nc = bacc.Bacc("TRN2", target_bir_lowering=False, debug=False,
                   num_devices=NCORES)

    # x tiles per half, ordered (j, u): tile 2j+u = rows j*128..j*128+128 of
    # chunk u, so the two chunks' j-th tiles sit adjacent for col-tiled agg.
    # x0g additionally carries the one-hot agg block g in its tail bytes.
    x0g_d = nc.dram_tensor("x0g", [128, 14 * D + NJ * CS], bf16,
                           kind="ExternalInput")
    x1_d = nc.dram_tensor("x1", [128, 14, D], bf16, kind="ExternalInput")
    xgT_d = nc.dram_tensor("xgT", [128, NT, KT, 128], fp8,
                           kind="ExternalInput")
    ws_d = nc.dram_tensor("ws", [128, KT, D], fp8, kind="ExternalInput")
    wn_d = nc.dram_tensor("wn", [128, KT, D], bf16, kind="ExternalInput")
    # aux packs the [64,64] transpose identity (cols 0-63) + bias row
    # (row 0, cols 64-575): both land at base partition 0
    aux_d = nc.dram_tensor("aux", [HS, HS + D], bf16, kind="ExternalInput")
    out_d = nc.dram_tensor("out", [NT, 128, D], bf16, kind="ExternalOutput")

    with tile.TileContext(nc) as tc:
        with (
            tc.tile_pool(name="const", bufs=1) as cpool,
            tc.tile_pool(name="tmp", bufs=3) as tpool,
            tc.tile_pool(name="outp", bufs=4) as opool,
            tc.tile_pool(name="chain", bufs=3,
                         space=bass.MemorySpace.PSUM) as chain,
            tc.tile_pool(name="mains", bufs=5,
                         space=bass.MemorySpace.PSUM) as mains,
        ):
            # ---- loads: sync ring, 8 DMAs in consumption order ----
            # weights stream on the sync ring; x streams concurrently on the
            # scalar ring (the 16 SDMA engines round-robin between rings),
            # so PE gets all W_self work early while x is still in flight.
            aux_sb = cpool.tile([HS, HS + D], bf16)
            nc.sync.dma_start(aux_sb[:], aux_d[:, :])
            ws_sb = cpool.tile([128, KT, D], fp8)
            nc.sync.dma_start(ws_sb[:], ws_d[:, :, :])
            xgT_a = cpool.tile([128, NPRE, KT, 128], fp8)
            nc.sync.dma_start(xgT_a[:], xgT_d[:, 0:NPRE, :, :])
            xgT_b = cpool.tile([128, NT - NPRE, KT, 128], fp8)
            nc.sync.dma_start(xgT_b[:], xgT_d[:, NPRE:NT, :, :])
            wn_sb = cpool.tile([128, KT, D], bf16)
            nc.sync.dma_start(wn_sb[:], wn_d[:, :, :])
            x0g_sb = cpool.tile([128, 14 * D + NJ * CS], bf16)
            nc.scalar.dma_start(x0g_sb[:], x0g_d[:, :])
            x1_sb = cpool.tile([128, 14, D], bf16)
            nc.scalar.dma_start(x1_sb[:, 0:8, :], x1_d[:, 0:8, :])
            nc.scalar.dma_start(x1_sb[:, 8:14, :], x1_d[:, 8:14, :])
            id_sb = aux_sb[0:HS, 0:HS]
            b_sb = aux_sb[0:1, HS:HS + D]

            def xtile(h, idx):
                if h == 0:
                    return x0g_sb[:, idx * D:(idx + 1) * D]
                return x1_sb[:, idx, :]

            def g_ap(j):
                return x0g_sb[:, 14 * D + j * CS:14 * D + (j + 1) * CS]
            ones_sb = cpool.tile([1, 128], bf16)
            nc.gpsimd.memset(ones_sb[:], 1.0)
            wml_sb = cpool.tile([128, 128], bf16)
            nc.gpsimd.memset(wml_sb[:], 1.0)
            wmr_sb = cpool.tile([128, D], bf16)
            nc.gpsimd.memset(wmr_sb[:], 1.0)

            def xgt2(t, kp):
                # [128, 2, 128] kt-pair slab for DoubleRow
                if t < NPRE:
                    return xgT_a[:, t, 2 * kp:2 * kp + 2, :]
                return xgT_b[:, t - NPRE, 2 * kp:2 * kp + 2, :]

            def open_group(t):
                ps = mains.tile([128, D], f32, tag="ps", name=f"ps_{t}")
                for kp in range(KT // 2):
                    nc.tensor.matmul(ps[:], xgt2(t, kp),
                                     ws_sb[:, 2 * kp:2 * kp + 2, :],
                                     start=(kp == 0), stop=(kp == 1),
                                     perf_mode=DR)
                return ps

            def stash(t, ps, eng):
                # immediate PSUM->SBUF evac (no A dependency): frees the bank
                z = cpool.tile([128, D], bf16, name=f"z_{t}")
                if eng == "act":
                    nc.scalar.copy(z[:], ps[:])
                else:
                    nc.vector.tensor_copy(z[:], ps[:])
                return z

            def agg_half(h):
                """aggregation of half h into two fresh PSUM banks"""
                psS = []
                for u in range(NCH):
                    psS_u = chain.tile([128, D], f32, tag="chain",
                                       name=f"psS_{h}_{u}")
                    psS.append(psS_u)
                for j in range(NJ):
                    for u in range(NCH):
                        nc.tensor.matmul(
                            psS[u][32 * u:32 * u + 32, :],
                            g_ap(j), xtile(h, 2 * j + u),
                            start=(j == 0), stop=(j == NJ - 1),
                            tile_position=(0, 32 * u),
                        )
                # psS -> SBUF on two engines in parallel (frees both banks)
                s_nat = cpool.tile([HS, D], bf16, name=f"s_nat_{h}")
                nc.vector.tensor_copy(s_nat[0:32, :], psS[0][0:32, :])
                nc.scalar.copy(s_nat[32:64, :], psS[1][32:64, :])
                return s_nat

            def tr_half(h, s_nat):
                """S^T (column-doubled) into SBUF"""
                psTr = chain.tile([128, KT, HS], bf16, tag="chain",
                                  name=f"psTr_{h}")
                for dt in range(KT):
                    nc.tensor.transpose(psTr[:, dt, :],
                                        s_nat[:, dt * 128:(dt + 1) * 128],
                                        id_sb)
                sT2 = cpool.tile([128, KT, 128], bf16, name=f"sT2_{h}")
                nc.vector.tensor_copy(sT2[:, :, 0:HS], psTr[:])
                nc.vector.tensor_copy(sT2[:, :, HS:128], psTr[:])
                return sT2

            def a_half(h, sT2):
                """A = S @ W_nbr + b; bias-matmul first (off the chain)"""
                psA = chain.tile([128, D], f32, tag="chain", name=f"psA_{h}")
                nc.tensor.matmul(psA[:], ones_sb[:1, :], b_sb,
                                 start=True, stop=False)
                for kt in range(KT):
                    nc.tensor.matmul(psA[:], sT2[:, kt, :], wn_sb[:, kt, :],
                                     start=False, stop=(kt == KT - 1))
                a2 = cpool.tile([128, D], bf16, name=f"a2_{h}")
                nc.vector.tensor_copy(a2[:], psA[:])
                return a2

            def finish(t, z, a2, add_eng, relu_eng):
                # z + A (both bf16 in SBUF), then relu, then store
                tmp = tpool.tile([128, D], bf16, tag="tmp")
                if add_eng == "gps":
                    nc.gpsimd.tensor_tensor(tmp[:], z[:], a2[:], op=Add)
                else:
                    nc.vector.tensor_tensor(tmp[:], z[:], a2[:], op=Add)
                ot = opool.tile([128, D], bf16, tag="ot")
                if relu_eng == "dve":
                    nc.vector.tensor_scalar_max(ot[:], tmp[:], 0.0)
                else:
                    nc.scalar.activation(ot[:], tmp[:], Relu)
                nc.sync.dma_start(out_d[t], ot[:])

            # ---- PE warm-up: dep-free FULL-ARRAY matmuls bridge the
            # ---- pre-data idle so the HAM clock gate reads the PE as busy
            # ---- and ungates to 2.4 GHz before real work arrives
            psW = chain.tile([128, D], f32, tag="chain", name="psW")
            for i in range(10):
                nc.tensor.matmul(psW[:], wml_sb[:], wmr_sb[:],
                                 start=(i == 0), stop=(i == 9))
            wsink = cpool.tile([1, D], bf16, name="wsink")
            nc.vector.tensor_copy(wsink[:], psW[0:1, :])

            # ---- emission in expected data-arrival order ----
            grp, zs = {}, {}
            for t in range(NT):
                grp[t] = open_group(t)
                zs[t] = stash(t, grp[t], "act" if t % 2 else "dve")
            s0 = agg_half(0)
            sT2_0 = tr_half(0, s0)
            s1 = agg_half(1)
            a2_0 = a_half(0, sT2_0)
            sT2_1 = tr_half(1, s1)
            a2_1 = a_half(1, sT2_1)
            # half-0 finishes ride GPSIMD+ACT (mid-kernel); half-1 finishes
            # are the tail: DVE adds, relu split DVE/ACT
            for t in range(5):
                finish(t, zs[t], a2_0, "gps", "act")
            for t in range(5, NT):
                finish(t, zs[t], a2_1, "dve", "dve" if t in (6, 8) else "act")

    nc.compile()
    return nc


def _get_compiled():
    global _compiled
    if _compiled is None:
        _compiled = _build_bass()
    return _compiled


def _rowl_table():
    """row_local[t, m]: xg row (s*10+r) held by partition m of out tile t."""
    t = np.arange(NT)[:, None]
    m = np.arange(128)[None, :]
    h, rp = t // 5, t % 5
    r = 2 * rp + m // HS
    s = HS * h + m % HS
    return (s * MAXR + r).astype(np.int64)


def _host_prep(inputs):
    """Shard + preprocess on host. Returns per-core input maps."""
    x = np.asarray(inputs["spatial_branch_feature_map"], dtype=np.float32)
    W_self = np.asarray(inputs["W_self"], dtype=np.float32)
    W_nbr = np.asarray(inputs["W_nbr"], dtype=np.float32)
    b = np.asarray(inputs["b"], dtype=np.float32)
    st = np.asarray(inputs["slicing_tensor"])
    op = np.asarray(inputs["object_pairs"])

    N = x.shape[0]
    n = NOBJ
    # exact replication of the reference's LUT-based row computation
    keys = st[:, 0].astype(np.int64) * (n * n) + st[:, 1].astype(np.int64) * n \
        + st[:, 2].astype(np.int64)
    lut = np.zeros(B * n * n, dtype=np.int64)
    lut[keys] = np.arange(N, dtype=np.int64)
    pmin = np.minimum(op[..., 0], op[..., 1]).astype(np.int64)
    pmax = np.maximum(op[..., 0], op[..., 1]).astype(np.int64)
    rel_keys = (np.arange(B, dtype=np.int64)[:, None] * (n * n)
                + pmin * n + pmax).reshape(-1)
    rows = lut[rel_keys]                      # [B*MAXR] global row index

    rowl = _rowl_table()                      # [NT, 128]

    # x: [NCORES, NH, 128, 14, D]; tile 2j+u = rows j*128.. of chunk 2h+u
    x_bf = (x.astype(BF16)
            .reshape(NCORES, NH, NCH, NJ, 128, D)      # [c, h, u, j, p, d]
            .transpose(0, 1, 4, 3, 2, 5)               # [c, h, p, j, u, d]
            .reshape(NCORES, NH, 128, 14 * D))
    x_bf = np.ascontiguousarray(x_bf)

    # xgT: [NCORES, 128, NT, KT, 128]; [p, t, kt, m] = xg[rowl[t,m], kt*128+p]
    xg = x[rows].astype(FP8).reshape(NCORES, ML, D)
    xgT = np.empty((NCORES, 128, NT, KT, 128), dtype=FP8)
    for c in range(NCORES):
        sel = xg[c][rowl.ravel()]             # [NT*128, D]
        xgT[c] = (sel.reshape(NT, 128, KT, 128)        # [t, m, kt, p]
                  .transpose(3, 0, 2, 1))              # [p, t, kt, m]
    xgT = np.ascontiguousarray(xgT)

    def wlay(W, dt):  # [D, D] -> [128, KT, D]: [p, kt, n] = W[kt*128+p, n]
        return np.ascontiguousarray(
            W.astype(dt).reshape(KT, 128, D).transpose(1, 0, 2))

    ws = wlay(W_self, FP8)
    wn = wlay(W_nbr, BF16)
    # one-hot agg block: g[p, j, s] = ((j*128 + p)//NC2 == s), s in [0, 32)
    jj = np.arange(NJ * 128)
    g = (jj[:, None] // NC2 == np.arange(CS)[None, :]).astype(BF16)
    g = np.ascontiguousarray(
        g.reshape(NJ, 128, CS).transpose(1, 0, 2))
    aux = np.zeros((HS, HS + D), dtype=BF16)
    aux[0:HS, 0:HS] = np.eye(HS, dtype=BF16)
    aux[0, HS:HS + D] = b.astype(BF16)

    in_maps = []
    for c in range(NCORES):
        x0g = np.concatenate([x_bf[c, 0], g.reshape(128, NJ * CS)], axis=1)
        in_maps.append({
            "x0g": np.ascontiguousarray(x0g),
            "x1": x_bf[c, 1].reshape(128, 14, D),
            "xgT": xgT[c], "ws": ws, "wn": wn, "aux": aux,
        })
    return in_maps


def _unpermute(out_cores):
    """[NCORES][NT, 128, D] bf16 -> [B*MAXR, D] f32 in reference order."""
    rowl = _rowl_table().ravel()
    out = np.empty((NCORES * ML, D), dtype=np.float32)
    for c in range(NCORES):
        oc = np.asarray(out_cores[c]).reshape(NT * 128, D)
        out[c * ML + rowl] = oc.astype(np.float32)
    return out


def run(inputs, trace=False):
    """Returns (full_output, BassKernelResults)."""
    from concourse.bass_utils import run_bass_kernel_spmd

    nc = _get_compiled()
    in_maps = _host_prep(inputs)
    res = run_bass_kernel_spmd(nc, in_maps, core_ids=list(range(NCORES)),
                               trace=trace)
    out = _unpermute([r["out"] for r in res.results])
    return out, res


def kernel(**inputs) -> np.ndarray:
    out, _ = run(inputs, trace=False)
    return out


# revision 34
# speedup vs baseline: 1.1266x; 1.1266x over previous
"""Trainium2 Bass kernel for the GraphicalBranch GNN message-passing problem.

Math (equivalent to the reference):
  - Per-sample graphs are fully connected WITH self-loops over the nc2=28
    pair-nodes, so segment_sum(x[src], dst) == broadcast of the per-sample
    row-sum S[b] = sum_r x[b, r, :].
  - The final key-matching gather h[rows] commutes with the row-wise linear
    layer, so only the 10 gathered rows per sample are pushed through W_self:
        out[row] = relu(xg[row] @ W_self + (S[b(row)] @ W_nbr + b))
  - rows are computed on host from slicing_tensor/object_pairs (pure index
    arithmetic), exactly as the reference's LUT does.

Device-side structure (per core: 128 samples, 3584 x-rows, 1280 out rows):
  - Output rows are regrouped into 10 tiles of 128 rows keyed (t = h*5+rp):
    partition m of tile t holds relation r=2*rp+m//64 of sample s=64h+m%64,
    which makes the neighbor term A[s] PARTITION-ALIGNED per tile: a plain
    DVE add. No expansion one-hots.
  - W_self matmuls run fp8 DoubleRow: each output tile is 2 matmuls with a
    fused 256-deep contraction (kt-pairs), ~1.5x PE throughput.
  - Every W_self PSUM group is stashed to SBUF (bf16) the moment it closes
    (no dependency on A), so the 5 PSUM "mains" banks recycle early and the
    finish (z + A -> relu -> store) runs in cheap bf16 ops.
  - Samples run in 2 halves of 64 (2 chunks of 32); aggregation pipelines
    through a bufs=3 PSUM chain pool (psS pair -> psTr -> psA per half);
    every bank is written, read once, freed - no bank read/write overlap.
  - Aggregation: one-hot G as lhsT, x as rhs; the two chunks of a half run
    as column-tiled concurrent matmuls (tile_position 0/32).
  - Dep-free full-array warm-up matmuls run before the first W_self group
    and in the pre-aggregation idle window so the PE HAM clock gate stays
    at 2.4 GHz (idle >3.4us re-throttles the PE to 1.2 GHz).
  - x/W_nbr travel bf16 (they feed the dominant S@W_nbr term); xgT/W_self
    travel fp8 e4m3 (<1% absmax error). Output bf16, host upcasts.
"""

import numpy as np
import ml_dtypes

# ---- problem constants (hardcoded; kernel.py must be self-contained) ----
B = 1024          # samples
NOBJ = 8          # objects per sample
NC2 = 28          # pair-nodes per sample
MAXR = 10         # relations per sample
D = 512           # feature dim
NCORES = 8
BL = B // NCORES          # 128 samples per core
RL = BL * NC2             # 3584 x-rows per core
ML = BL * MAXR            # 1280 output rows per core
KT = D // 128             # 4 contraction tiles
NH = 2                    # sample halves per core
HS = BL // NH             # 64 samples per half
NCH = 2                   # chunks per half
CS = HS // NCH            # 32 samples per chunk
NJ = 7                    # 128-row tiles per chunk (896 rows)
NT = 10                   # output tiles per core
NPRE = 5                  # xgT tiles prefetched before x

BF16 = ml_dtypes.bfloat16
FP8 = ml_dtypes.float8_e4m3

_compiled = None


def _build_bass():
    import concourse.bacc as bacc
    import concourse.bass as bass
    import concourse.mybir as mybir
    from concourse import tile

    f32 = mybir.dt.float32
    bf16 = mybir.dt.bfloat16
    fp8 = mybir.dt.float8e4
    Relu = mybir.ActivationFunctionType.Relu
    Add = mybir.AluOpType.add
    DR = mybir.MatmulPerfMode.DoubleRow

    nc = bacc.Bacc("TRN2", target_bir_lowering=False, debug=False,
                   num_devices=NCORES)

    # x tiles per half, ordered (j, u): tile 2j+u = rows j*128..j*128+128 of
    # chunk u, so the two chunks' j-th tiles sit adjacent for col-tiled agg.
    x_d = nc.dram_tensor("x", [NH, 128, 14, D], bf16, kind="ExternalInput")
    g_d = nc.dram_tensor("g", [128, NJ, CS], bf16, kind="ExternalInput")
    xgT_d = nc.dram_tensor("xgT", [128, NT, KT, 128], fp8,
                           kind="ExternalInput")
    ws_d = nc.dram_tensor("ws", [128, KT, D], fp8, kind="ExternalInput")
    wn_d = nc.dram_tensor("wn", [128, KT, D], bf16, kind="ExternalInput")
    b_d = nc.dram_tensor("bias", [1, D], bf16, kind="ExternalInput")
    id_d = nc.dram_tensor("ident", [HS, HS], bf16, kind="ExternalInput")
    out_d = nc.dram_tensor("out", [NT, 128, D], bf16, kind="ExternalOutput")

    with tile.TileContext(nc) as tc:
        with (
            tc.tile_pool(name="const", bufs=1) as cpool,
            tc.tile_pool(name="tmp", bufs=3) as tpool,
            tc.tile_pool(name="outp", bufs=4) as opool,
            tc.tile_pool(name="chain", bufs=3,
                         space=bass.MemorySpace.PSUM) as chain,
            tc.tile_pool(name="mains", bufs=5,
                         space=bass.MemorySpace.PSUM) as mains,
        ):
            # ---- loads: sync ring, 8 DMAs in consumption order ----
            ws_sb = cpool.tile([128, KT, D], fp8)
            nc.sync.dma_start(ws_sb[:], ws_d[:, :, :])
            xgT_a = cpool.tile([128, NPRE, KT, 128], fp8)
            nc.sync.dma_start(xgT_a[:], xgT_d[:, 0:NPRE, :, :])
            g_sb = cpool.tile([128, NJ, CS], bf16)
            nc.sync.dma_start(g_sb[:], g_d[:, :, :])
            x_sb = []
            for h in range(NH):
                xh = cpool.tile([128, 14, D], bf16, name=f"x_sb_{h}")
                x_sb.append(xh)
            nc.sync.dma_start(x_sb[0][:], x_d[0])
            wn_sb = cpool.tile([128, KT, D], bf16)
            nc.sync.dma_start(wn_sb[:], wn_d[:, :, :])
            nc.sync.dma_start(x_sb[1][:, 0:8, :], x_d[1][:, 0:8, :])
            nc.sync.dma_start(x_sb[1][:, 8:14, :], x_d[1][:, 8:14, :])
            xgT_b = cpool.tile([128, NT - NPRE, KT, 128], fp8)
            nc.sync.dma_start(xgT_b[:], xgT_d[:, NPRE:NT, :, :])

            # ---- small loads on the scalar ring ----
            id_sb = cpool.tile([HS, HS], bf16)
            nc.scalar.dma_start(id_sb[:], id_d[:, :])
            b_sb = cpool.tile([1, D], bf16)
            nc.scalar.dma_start(b_sb[:], b_d[:, :])
            ones_sb = cpool.tile([1, 128], bf16)
            nc.gpsimd.memset(ones_sb[:], 1.0)
            wml_sb = cpool.tile([128, 128], bf16)
            nc.gpsimd.memset(wml_sb[:], 1.0)
            wmr_sb = cpool.tile([128, D], bf16)
            nc.gpsimd.memset(wmr_sb[:], 1.0)

            def warmup(n, tag):
                psW = chain.tile([128, D], f32, tag="chain", name=f"psW{tag}")
                for i in range(n):
                    nc.tensor.matmul(psW[:], wml_sb[:], wmr_sb[:],
                                     start=(i == 0), stop=(i == n - 1))
                sink = cpool.tile([1, D], bf16, name=f"wsink{tag}")
                nc.vector.tensor_copy(sink[:], psW[0:1, :])

            def xgt2(t, kp):
                # [128, 2, 128] kt-pair slab for DoubleRow
                if t < NPRE:
                    return xgT_a[:, t, 2 * kp:2 * kp + 2, :]
                return xgT_b[:, t - NPRE, 2 * kp:2 * kp + 2, :]

            def open_group(t):
                ps = mains.tile([128, D], f32, tag="ps", name=f"ps_{t}")
                for kp in range(KT // 2):
                    nc.tensor.matmul(ps[:], xgt2(t, kp),
                                     ws_sb[:, 2 * kp:2 * kp + 2, :],
                                     start=(kp == 0), stop=(kp == 1),
                                     perf_mode=DR)
                return ps

            def stash(t, ps, eng):
                # immediate PSUM->SBUF evac (no A dependency): frees the bank
                z = cpool.tile([128, D], bf16, name=f"z_{t}")
                if eng == "act":
                    nc.scalar.copy(z[:], ps[:])
                else:
                    nc.vector.tensor_copy(z[:], ps[:])
                return z

            def agg_half(h):
                """aggregation of half h into two fresh PSUM banks"""
                psS = []
                for u in range(NCH):
                    psS_u = chain.tile([128, D], f32, tag="chain",
                                       name=f"psS_{h}_{u}")
                    psS.append(psS_u)
                for j in range(NJ):
                    for u in range(NCH):
                        nc.tensor.matmul(
                            psS[u][32 * u:32 * u + 32, :],
                            g_sb[:, j, :], x_sb[h][:, 2 * j + u, :],
                            start=(j == 0), stop=(j == NJ - 1),
                            tile_position=(0, 32 * u),
                        )
                # psS -> SBUF on two engines in parallel (frees both banks)
                s_nat = cpool.tile([HS, D], bf16, name=f"s_nat_{h}")
                nc.vector.tensor_copy(s_nat[0:32, :], psS[0][0:32, :])
                nc.scalar.copy(s_nat[32:64, :], psS[1][32:64, :])
                return s_nat

            def tr_half(h, s_nat):
                """S^T (column-doubled) into SBUF"""
                psTr = chain.tile([128, KT, HS], bf16, tag="chain",
                                  name=f"psTr_{h}")
                for dt in range(KT):
                    nc.tensor.transpose(psTr[:, dt, :],
                                        s_nat[:, dt * 128:(dt + 1) * 128],
                                        id_sb[:, :])
                sT2 = cpool.tile([128, KT, 128], bf16, name=f"sT2_{h}")
                nc.vector.tensor_copy(sT2[:, :, 0:HS], psTr[:])
                nc.vector.tensor_copy(sT2[:, :, HS:128], psTr[:])
                return sT2

            def a_half(h, sT2):
                """A = S @ W_nbr + b; bias-matmul first (off the chain)"""
                psA = chain.tile([128, D], f32, tag="chain", name=f"psA_{h}")
                nc.tensor.matmul(psA[:], ones_sb[:1, :], b_sb[:],
                                 start=True, stop=False)
                for kt in range(KT):
                    nc.tensor.matmul(psA[:], sT2[:, kt, :], wn_sb[:, kt, :],
                                     start=False, stop=(kt == KT - 1))
                a2 = cpool.tile([128, D], bf16, name=f"a2_{h}")
                nc.vector.tensor_copy(a2[:], psA[:])
                return a2

            def finish(t, z, a2):
                # z + A (bf16 2x) on DVE; relu alternates ACT/DVE; store
                tmp = tpool.tile([128, D], bf16, tag="tmp")
                nc.vector.tensor_tensor(tmp[:], z[:], a2[:], op=Add)
                ot = opool.tile([128, D], bf16, tag="ot")
                if t in (1, 3, 5):
                    nc.vector.tensor_scalar_max(ot[:], tmp[:], 0.0)
                else:
                    nc.scalar.activation(ot[:], tmp[:], Relu)
                nc.sync.dma_start(out_d[t], ot[:])

            # ---- emission in expected data-arrival order ----
            warmup(10, "a")
            grp, zs = {}, {}
            for t in range(5):
                grp[t] = open_group(t)
                zs[t] = stash(t, grp[t], "act" if t % 2 else "dve")
            # bridge the G0-4 -> agg idle window (keeps HAM warm)
            warmup(10, "b")
            s0 = agg_half(0)
            s1 = agg_half(1)
            sT2_0 = tr_half(0, s0)
            sT2_1 = tr_half(1, s1)
            a2_0 = a_half(0, sT2_0)
            for t in (5, 6):
                grp[t] = open_group(t)
                zs[t] = stash(t, grp[t], "act" if t % 2 else "dve")
            a2_1 = a_half(1, sT2_1)
            for t in (7, 8, 9):
                grp[t] = open_group(t)
                zs[t] = stash(t, grp[t], "act" if t % 2 else "dve")
            for t in range(5):
                finish(t, zs[t], a2_0)
            for t in range(5, NT):
                finish(t, zs[t], a2_1)

    nc.compile()
    return nc


def _get_compiled():
    global _compiled
    if _compiled is None:
        _compiled = _build_bass()
    return _compiled


def _rowl_table():
    """row_local[t, m]: xg row (s*10+r) held by partition m of out tile t."""
    t = np.arange(NT)[:, None]
    m = np.arange(128)[None, :]
    h, rp = t // 5, t % 5
    r = 2 * rp + m // HS
    s = HS * h + m % HS
    return (s * MAXR + r).astype(np.int64)


def _host_prep(inputs):
    """Shard + preprocess on host. Returns per-core input maps."""
    x = np.asarray(inputs["spatial_branch_feature_map"], dtype=np.float32)
    W_self = np.asarray(inputs["W_self"], dtype=np.float32)
    W_nbr = np.asarray(inputs["W_nbr"], dtype=np.float32)
    b = np.asarray(inputs["b"], dtype=np.float32)
    st = np.asarray(inputs["slicing_tensor"])
    op = np.asarray(inputs["object_pairs"])

    N = x.shape[0]
    n = NOBJ
    # exact replication of the reference's LUT-based row computation
    keys = st[:, 0].astype(np.int64) * (n * n) + st[:, 1].astype(np.int64) * n \
        + st[:, 2].astype(np.int64)
    lut = np.zeros(B * n * n, dtype=np.int64)
    lut[keys] = np.arange(N, dtype=np.int64)
    pmin = np.minimum(op[..., 0], op[..., 1]).astype(np.int64)
    pmax = np.maximum(op[..., 0], op[..., 1]).astype(np.int64)
    rel_keys = (np.arange(B, dtype=np.int64)[:, None] * (n * n)
                + pmin * n + pmax).reshape(-1)
    rows = lut[rel_keys]                      # [B*MAXR] global row index

    rowl = _rowl_table()                      # [NT, 128]

    # x: [NCORES, NH, 128, 14, D]; tile 2j+u = rows j*128.. of chunk 2h+u
    x_bf = (x.astype(BF16)
            .reshape(NCORES, NH, NCH, NJ, 128, D)      # [c, h, u, j, p, d]
            .transpose(0, 1, 4, 3, 2, 5)               # [c, h, p, j, u, d]
            .reshape(NCORES, NH, 128, 14, D))
    x_bf = np.ascontiguousarray(x_bf)

    # xgT: [NCORES, 128, NT, KT, 128]; [p, t, kt, m] = xg[rowl[t,m], kt*128+p]
    xg = x[rows].astype(FP8).reshape(NCORES, ML, D)
    xgT = np.empty((NCORES, 128, NT, KT, 128), dtype=FP8)
    for c in range(NCORES):
        sel = xg[c][rowl.ravel()]             # [NT*128, D]
        xgT[c] = (sel.reshape(NT, 128, KT, 128)        # [t, m, kt, p]
                  .transpose(3, 0, 2, 1))              # [p, t, kt, m]
    xgT = np.ascontiguousarray(xgT)

    def wlay(W, dt):  # [D, D] -> [128, KT, D]: [p, kt, n] = W[kt*128+p, n]
        return np.ascontiguousarray(
            W.astype(dt).reshape(KT, 128, D).transpose(1, 0, 2))

    ws = wlay(W_self, FP8)
    wn = wlay(W_nbr, BF16)
    # one-hot agg block: g[p, j, s] = ((j*128 + p)//NC2 == s), s in [0, 32)
    jj = np.arange(NJ * 128)
    g = (jj[:, None] // NC2 == np.arange(CS)[None, :]).astype(BF16)
    g = np.ascontiguousarray(
        g.reshape(NJ, 128, CS).transpose(1, 0, 2))
    bias = b.astype(BF16).reshape(1, D)
    ident = np.eye(HS, dtype=BF16)

    in_maps = []
    for c in range(NCORES):
        in_maps.append({
            "x": x_bf[c], "xgT": xgT[c], "g": g,
            "ws": ws, "wn": wn, "bias": bias, "ident": ident,
        })
    return in_maps


def _unpermute(out_cores):
    """[NCORES][NT, 128, D] bf16 -> [B*MAXR, D] f32 in reference order."""
    rowl = _rowl_table().ravel()
    out = np.empty((NCORES * ML, D), dtype=np.float32)
    for c in range(NCORES):
        oc = np.asarray(out_cores[c]).reshape(NT * 128, D)
        out[c * ML + rowl] = oc.astype(np.float32)
    return out


def run(inputs, trace=False):
    """Returns (full_output, BassKernelResults)."""
    from concourse.bass_utils import run_bass_kernel_spmd

    nc = _get_compiled()
    in_maps = _host_prep(inputs)
    res = run_bass_kernel_spmd(nc, in_maps, core_ids=list(range(NCORES)),
                               trace=trace)
    out = _unpermute([r["out"] for r in res.results])
    return out, res


def kernel(**inputs) -> np.ndarray:
    out, _ = run(inputs, trace=False)
    return out


# revision 37
# speedup vs baseline: 1.2010x; 1.0660x over previous
"""Trainium2 Bass kernel for the GraphicalBranch GNN message-passing problem.

Math (equivalent to the reference):
  - Per-sample graphs are fully connected WITH self-loops over the nc2=28
    pair-nodes, so segment_sum(x[src], dst) == broadcast of the per-sample
    row-sum S[b] = sum_r x[b, r, :].
  - The final key-matching gather h[rows] commutes with the row-wise linear
    layer, so only the 10 gathered rows per sample are pushed through W_self:
        out[row] = relu(xg[row] @ W_self + (S[b(row)] @ W_nbr + b))
  - rows are computed on host from slicing_tensor/object_pairs (pure index
    arithmetic), exactly as the reference's LUT does.

Device-side structure (per core: 128 samples, 3584 x-rows, 1280 out rows):
  - Output rows are regrouped into 10 tiles of 128 rows keyed (t = h*5+rp):
    partition m of tile t holds relation r=2*rp+m//64 of sample s=64h+m%64,
    which makes the neighbor term A[s] PARTITION-ALIGNED per tile: a plain
    DVE add. No expansion one-hots.
  - W_self matmuls run fp8 DoubleRow: each output tile is 2 matmuls with a
    fused 256-deep contraction (kt-pairs), ~1.5x PE throughput.
  - Every W_self PSUM group is stashed to SBUF (bf16) the moment it closes
    (no dependency on A), so the 5 PSUM "mains" banks recycle early and the
    finish (z + A -> relu -> store) runs in cheap bf16 ops.
  - Samples run in 2 halves of 64 (2 chunks of 32); aggregation pipelines
    through a bufs=3 PSUM chain pool (psS pair -> psTr -> psA per half);
    every bank is written, read once, freed - no bank read/write overlap.
  - Aggregation: one-hot G as lhsT, x as rhs; the two chunks of a half run
    as column-tiled concurrent matmuls (tile_position 0/32).
  - x/W_nbr travel bf16 (they feed the dominant S@W_nbr term); xgT/W_self
    travel fp8 e4m3 (<1% absmax error). Output bf16, host upcasts.
"""

import numpy as np
import ml_dtypes

# ---- problem constants (hardcoded; kernel.py must be self-contained) ----
B = 1024          # samples
NOBJ = 8          # objects per sample
NC2 = 28          # pair-nodes per sample
MAXR = 10         # relations per sample
D = 512           # feature dim
NCORES = 8
BL = B // NCORES          # 128 samples per core
RL = BL * NC2             # 3584 x-rows per core
ML = BL * MAXR            # 1280 output rows per core
KT = D // 128             # 4 contraction tiles
NH = 2                    # sample halves per core
HS = BL // NH             # 64 samples per half
NCH = 2                   # chunks per half
CS = HS // NCH            # 32 samples per chunk
NJ = 7                    # 128-row tiles per chunk (896 rows)
NT = 10                   # output tiles per core
NPRE = 5                  # xgT tiles prefetched before x

BF16 = ml_dtypes.bfloat16
FP8 = ml_dtypes.float8_e4m3

_compiled = None


def _build_bass():
    import concourse.bacc as bacc
    import concourse.bass as bass
    import concourse.mybir as mybir
    from concourse import tile

    f32 = mybir.dt.float32
    bf16 = mybir.dt.bfloat16
    fp8 = mybir.dt.float8e4
    Relu = mybir.ActivationFunctionType.Relu
    Add = mybir.AluOpType.add
    DR = mybir.MatmulPerfMode.DoubleRow

    nc = bacc.Bacc("TRN2", target_bir_lowering=False, debug=False,
                   num_devices=NCORES)

    # x tiles per half, ordered (j, u): tile 2j+u = rows j*128..j*128+128 of
    # chunk u, so the two chunks' j-th tiles sit adjacent for col-tiled agg.
    x_d = nc.dram_tensor("x", [NH, 128, 14, D], bf16, kind="ExternalInput")
    g_d = nc.dram_tensor("g", [128, NJ, CS], bf16, kind="ExternalInput")
    xgT_d = nc.dram_tensor("xgT", [128, NT, KT, 128], fp8,
                           kind="ExternalInput")
    ws_d = nc.dram_tensor("ws", [128, KT, D], fp8, kind="ExternalInput")
    wn_d = nc.dram_tensor("wn", [128, KT, D], bf16, kind="ExternalInput")
    b_d = nc.dram_tensor("bias", [1, D], bf16, kind="ExternalInput")
    id_d = nc.dram_tensor("ident", [HS, HS], bf16, kind="ExternalInput")
    out_d = nc.dram_tensor("out", [NT, 128, D], bf16, kind="ExternalOutput")

    with tile.TileContext(nc) as tc:
        with (
            tc.tile_pool(name="const", bufs=1) as cpool,
            tc.tile_pool(name="tmp", bufs=3) as tpool,
            tc.tile_pool(name="outp", bufs=4) as opool,
            tc.tile_pool(name="chain", bufs=3,
                         space=bass.MemorySpace.PSUM) as chain,
            tc.tile_pool(name="mains", bufs=5,
                         space=bass.MemorySpace.PSUM) as mains,
        ):
            # ---- loads: sync ring, 8 DMAs in consumption order ----
            ws_sb = cpool.tile([128, KT, D], fp8)
            nc.sync.dma_start(ws_sb[:], ws_d[:, :, :])
            xgT_a = cpool.tile([128, NPRE, KT, 128], fp8)
            nc.sync.dma_start(xgT_a[:], xgT_d[:, 0:NPRE, :, :])
            g_sb = cpool.tile([128, NJ, CS], bf16)
            nc.sync.dma_start(g_sb[:], g_d[:, :, :])
            x_sb = []
            for h in range(NH):
                xh = cpool.tile([128, 14, D], bf16, name=f"x_sb_{h}")
                x_sb.append(xh)
            nc.sync.dma_start(x_sb[0][:], x_d[0])
            wn_sb = cpool.tile([128, KT, D], bf16)
            nc.sync.dma_start(wn_sb[:], wn_d[:, :, :])
            nc.sync.dma_start(x_sb[1][:, 0:8, :], x_d[1][:, 0:8, :])
            nc.sync.dma_start(x_sb[1][:, 8:14, :], x_d[1][:, 8:14, :])
            xgT_b = cpool.tile([128, NT - NPRE, KT, 128], fp8)
            nc.sync.dma_start(xgT_b[:], xgT_d[:, NPRE:NT, :, :])

            # ---- small loads on the scalar ring ----
            id_sb = cpool.tile([HS, HS], bf16)
            nc.scalar.dma_start(id_sb[:], id_d[:, :])
            b_sb = cpool.tile([1, D], bf16)
            nc.scalar.dma_start(b_sb[:], b_d[:, :])
            ones_sb = cpool.tile([1, 128], bf16)
            nc.gpsimd.memset(ones_sb[:], 1.0)

            def xgt2(t, kp):
                # [128, 2, 128] kt-pair slab for DoubleRow
                if t < NPRE:
                    return xgT_a[:, t, 2 * kp:2 * kp + 2, :]
                return xgT_b[:, t - NPRE, 2 * kp:2 * kp + 2, :]

            def open_group(t):
                ps = mains.tile([128, D], f32, tag="ps", name=f"ps_{t}")
                for kp in range(KT // 2):
                    nc.tensor.matmul(ps[:], xgt2(t, kp),
                                     ws_sb[:, 2 * kp:2 * kp + 2, :],
                                     start=(kp == 0), stop=(kp == 1),
                                     perf_mode=DR)
                return ps

            def stash(t, ps, eng):
                # immediate PSUM->SBUF evac (no A dependency): frees the bank
                z = cpool.tile([128, D], bf16, name=f"z_{t}")
                if eng == "act":
                    nc.scalar.copy(z[:], ps[:])
                else:
                    nc.vector.tensor_copy(z[:], ps[:])
                return z

            def agg_half(h):
                """aggregation of half h into two fresh PSUM banks"""
                psS = []
                for u in range(NCH):
                    psS_u = chain.tile([128, D], f32, tag="chain",
                                       name=f"psS_{h}_{u}")
                    psS.append(psS_u)
                for j in range(NJ):
                    for u in range(NCH):
                        nc.tensor.matmul(
                            psS[u][32 * u:32 * u + 32, :],
                            g_sb[:, j, :], x_sb[h][:, 2 * j + u, :],
                            start=(j == 0), stop=(j == NJ - 1),
                            tile_position=(0, 32 * u),
                        )
                # psS -> SBUF on two engines in parallel (frees both banks)
                s_nat = cpool.tile([HS, D], bf16, name=f"s_nat_{h}")
                nc.vector.tensor_copy(s_nat[0:32, :], psS[0][0:32, :])
                nc.scalar.copy(s_nat[32:64, :], psS[1][32:64, :])
                return s_nat

            def tr_half(h, s_nat):
                """S^T (column-doubled) into SBUF"""
                psTr = chain.tile([128, KT, HS], bf16, tag="chain",
                                  name=f"psTr_{h}")
                for dt in range(KT):
                    nc.tensor.transpose(psTr[:, dt, :],
                                        s_nat[:, dt * 128:(dt + 1) * 128],
                                        id_sb[:, :])
                sT2 = cpool.tile([128, KT, 128], bf16, name=f"sT2_{h}")
                nc.vector.tensor_copy(sT2[:, :, 0:HS], psTr[:])
                nc.vector.tensor_copy(sT2[:, :, HS:128], psTr[:])
                return sT2

            def a_half(h, sT2):
                """A = S @ W_nbr + b; bias-matmul first (off the chain)"""
                psA = chain.tile([128, D], f32, tag="chain", name=f"psA_{h}")
                nc.tensor.matmul(psA[:], ones_sb[:1, :], b_sb[:],
                                 start=True, stop=False)
                for kt in range(KT):
                    nc.tensor.matmul(psA[:], sT2[:, kt, :], wn_sb[:, kt, :],
                                     start=False, stop=(kt == KT - 1))
                a2 = cpool.tile([128, D], bf16, name=f"a2_{h}")
                nc.vector.tensor_copy(a2[:], psA[:])
                return a2

            def finish(t, z, a2):
                # z + A (bf16 2x) on DVE; relu alternates ACT/DVE; store
                tmp = tpool.tile([128, D], bf16, tag="tmp")
                nc.vector.tensor_tensor(tmp[:], z[:], a2[:], op=Add)
                ot = opool.tile([128, D], bf16, tag="ot")
                if t in (1, 3, 5):
                    nc.vector.tensor_scalar_max(ot[:], tmp[:], 0.0)
                else:
                    nc.scalar.activation(ot[:], tmp[:], Relu)
                nc.sync.dma_start(out_d[t], ot[:])

            # ---- emission in expected data-arrival order ----
            grp, zs = {}, {}
            for t in range(5):
                grp[t] = open_group(t)
                zs[t] = stash(t, grp[t], "act" if t % 2 else "dve")
            s0 = agg_half(0)
            s1 = agg_half(1)
            sT2_0 = tr_half(0, s0)
            sT2_1 = tr_half(1, s1)
            a2_0 = a_half(0, sT2_0)
            for t in (5, 6):
                grp[t] = open_group(t)
                zs[t] = stash(t, grp[t], "act" if t % 2 else "dve")
            a2_1 = a_half(1, sT2_1)
            for t in (7, 8, 9):
                grp[t] = open_group(t)
                zs[t] = stash(t, grp[t], "act" if t % 2 else "dve")
            for t in range(5):
                finish(t, zs[t], a2_0)
            for t in range(5, NT):
                finish(t, zs[t], a2_1)

    nc.compile()
    return nc


def _get_compiled():
    global _compiled
    if _compiled is None:
        _compiled = _build_bass()
    return _compiled


def _rowl_table():
    """row_local[t, m]: xg row (s*10+r) held by partition m of out tile t."""
    t = np.arange(NT)[:, None]
    m = np.arange(128)[None, :]
    h, rp = t // 5, t % 5
    r = 2 * rp + m // HS
    s = HS * h + m % HS
    return (s * MAXR + r).astype(np.int64)


def _host_prep(inputs):
    """Shard + preprocess on host. Returns per-core input maps."""
    x = np.asarray(inputs["spatial_branch_feature_map"], dtype=np.float32)
    W_self = np.asarray(inputs["W_self"], dtype=np.float32)
    W_nbr = np.asarray(inputs["W_nbr"], dtype=np.float32)
    b = np.asarray(inputs["b"], dtype=np.float32)
    st = np.asarray(inputs["slicing_tensor"])
    op = np.asarray(inputs["object_pairs"])

    N = x.shape[0]
    n = NOBJ
    # exact replication of the reference's LUT-based row computation
    keys = st[:, 0].astype(np.int64) * (n * n) + st[:, 1].astype(np.int64) * n \
        + st[:, 2].astype(np.int64)
    lut = np.zeros(B * n * n, dtype=np.int64)
    lut[keys] = np.arange(N, dtype=np.int64)
    pmin = np.minimum(op[..., 0], op[..., 1]).astype(np.int64)
    pmax = np.maximum(op[..., 0], op[..., 1]).astype(np.int64)
    rel_keys = (np.arange(B, dtype=np.int64)[:, None] * (n * n)
                + pmin * n + pmax).reshape(-1)
    rows = lut[rel_keys]                      # [B*MAXR] global row index

    rowl = _rowl_table()                      # [NT, 128]

    # x: [NCORES, NH, 128, 14, D]; tile 2j+u = rows j*128.. of chunk 2h+u
    x_bf = (x.astype(BF16)
            .reshape(NCORES, NH, NCH, NJ, 128, D)      # [c, h, u, j, p, d]
            .transpose(0, 1, 4, 3, 2, 5)               # [c, h, p, j, u, d]
            .reshape(NCORES, NH, 128, 14, D))
    x_bf = np.ascontiguousarray(x_bf)

    # xgT: [NCORES, 128, NT, KT, 128]; [p, t, kt, m] = xg[rowl[t,m], kt*128+p]
    xg = x[rows].astype(FP8).reshape(NCORES, ML, D)
    xgT = np.empty((NCORES, 128, NT, KT, 128), dtype=FP8)
    for c in range(NCORES):
        sel = xg[c][rowl.ravel()]             # [NT*128, D]
        xgT[c] = (sel.reshape(NT, 128, KT, 128)        # [t, m, kt, p]
                  .transpose(3, 0, 2, 1))              # [p, t, kt, m]
    xgT = np.ascontiguousarray(xgT)

    def wlay(W, dt):  # [D, D] -> [128, KT, D]: [p, kt, n] = W[kt*128+p, n]
        return np.ascontiguousarray(
            W.astype(dt).reshape(KT, 128, D).transpose(1, 0, 2))

    ws = wlay(W_self, FP8)
    wn = wlay(W_nbr, BF16)
    # one-hot agg block: g[p, j, s] = ((j*128 + p)//NC2 == s), s in [0, 32)
    jj = np.arange(NJ * 128)
    g = (jj[:, None] // NC2 == np.arange(CS)[None, :]).astype(BF16)
    g = np.ascontiguousarray(
        g.reshape(NJ, 128, CS).transpose(1, 0, 2))
    bias = b.astype(BF16).reshape(1, D)
    ident = np.eye(HS, dtype=BF16)

    in_maps = []
    for c in range(NCORES):
        in_maps.append({
            "x": x_bf[c], "xgT": xgT[c], "g": g,
            "ws": ws, "wn": wn, "bias": bias, "ident": ident,
        })
    return in_maps


def _unpermute(out_cores):
    """[NCORES][NT, 128, D] bf16 -> [B*MAXR, D] f32 in reference order."""
    rowl = _rowl_table().ravel()
    out = np.empty((NCORES * ML, D), dtype=np.float32)
    for c in range(NCORES):
        oc = np.asarray(out_cores[c]).reshape(NT * 128, D)
        out[c * ML + rowl] = oc.astype(np.float32)
    return out


def run(inputs, trace=False):
    """Returns (full_output, BassKernelResults)."""
    from concourse.bass_utils import run_bass_kernel_spmd

    nc = _get_compiled()
    in_maps = _host_prep(inputs)
    res = run_bass_kernel_spmd(nc, in_maps, core_ids=list(range(NCORES)),
                               trace=trace)
    out = _unpermute([r["out"] for r in res.results])
    return out, res


def kernel(**inputs) -> np.ndarray:
    out, _ = run(inputs, trace=False)
    return out
